# revision 11
# baseline (speedup 1.0000x reference)
"""Trainium2 Bass kernel for nn_Conv2d (B=32, Cin=Cout=64, H=W=112, 3x3, pad 1).

Strategy (tap-packed, full 128x128 PE utilization):
- Data-parallel: 32 images / 8 cores = 4 images per core; weights/bias replicated.
- Per image: SBUF partitions 0-63 hold the zero-padded (114x114, flattened) image;
  partitions 64-127 hold the SAME image shifted down one padded row (built by a
  contiguous SBUF->SBUF DMA). A matmul with K=128 then sees two vertically
  adjacent padded rows of the image at once.
- Per 456-element chunk (4 padded rows) only THREE matmuls (horizontal taps
  a=0,1,2), each with stationary lhsT [128,128]:
      [[w(0,a)^T, w(2,a)^T], [w(1,a)^T, 0]]
  PSUM partitions 0-63 accumulate P = rows-0/1 taps; partitions 64-127
  accumulate Q = row-2 taps on the unshifted grid.
- Epilogue: ScalarE copies Q to an SBUF ring (Qbuf); VectorE computes
  out = (P + bias) + Q[b + 2*114] with one fused scalar_tensor_tensor, dropping
  the pad columns so the staging buffer is in final HBM layout; outputs stream
  to HBM as contiguous row-block DMAs.
- dtype float32r: fp32 storage, fast PE mode (1 cycle/row at N>=256),
  ~1.4e-4 max relative error end-to-end.
"""
import numpy as np

B, CIN, COUT, H, W = 32, 64, 64, 112, 112
N_CORES = 8
IPC = B // N_CORES          # images per core = 4
Wp = W + 2                  # padded width 114
Hp = H + 2                  # padded height 114
RPC = 4                     # rows per chunk
CH = RPC * Wp               # chunk size 456 (row-aligned; fits one PSUM bank)
NCH = H // RPC              # 28 combine-chunks cover all 112 output rows
NCHQ = NCH + 1              # +1 chunk so Q is staged past the last output row
LB = 13440                  # padded-image buffer length (with tail margin)
RB = 28                     # rows per input/output DMA block

_CACHE = {}


def _build_module():
    import concourse.tile as tile
    from concourse import bacc, mybir
    from concourse.bass_interp import get_hw_module

    f32 = mybir.dt.float32
    f32r = mybir.dt.float32r
    ALU = mybir.AluOpType

    nc = bacc.Bacc("TRN2", target_bir_lowering=False, debug=False,
                   enable_asserts=False, num_devices=N_CORES)
    x_ap = nc.dram_tensor("x", [IPC, CIN, H, W], f32r, kind="ExternalInput").ap()
    wt_ap = nc.dram_tensor("wt", [3, 128, 128], f32r, kind="ExternalInput").ap()
    b_ap = nc.dram_tensor("bias2", [128, 1], f32, kind="ExternalInput").ap()
    y_ap = nc.dram_tensor("y", [IPC, COUT, H, W], f32, kind="ExternalOutput").ap()

    with tile.TileContext(nc) as tc:
        with (
            tc.tile_pool(name="const", bufs=1) as cp,
            tc.tile_pool(name="x2", bufs=2) as xp,
            tc.tile_pool(name="qo", bufs=1) as qp,
            tc.tile_pool(name="psum", bufs=6, space="PSUM") as pp,
        ):
            w_sb = cp.tile([128, 3 * 128], f32r)
            nc.sync.dma_start(w_sb[:].rearrange("k (t m) -> k t m", t=3),
                              wt_ap.rearrange("t k m -> k t m"))
            bias_sb = cp.tile([128, 1], f32)
            nc.sync.dma_start(bias_sb[:], b_ap[:])

            for img in range(IPC):
                x2 = xp.tile([128, LB], f32r)
                # zero pad borders on the top half (interior comes from DMA;
                # bottom half is duplicated from the top after)
                top = x2[0:64, :]
                nc.vector.memset(top[:, 0:Wp].bitcast(f32), 0.0)           # pad row 0
                nc.vector.memset(top[:, (Hp - 1) * Wp:LB].bitcast(f32), 0.0)  # row 113+tail
                interior = top[:, Wp:Wp + H * Wp].rearrange("p (h w) -> p h w", w=Wp)
                nc.vector.memset(interior[:, :, 0:1].bitcast(f32), 0.0)    # left pad col
                nc.vector.memset(interior[:, :, Wp - 1:Wp].bitcast(f32), 0.0)  # right pad
                # interior row-blocks, alternating Sync/GpSimd DMA queues
                for bi, rb in enumerate(range(0, H, RB)):
                    lo = Wp * (1 + rb) + 1
                    dst = top[:, lo:lo + RB * Wp]
                    dst = dst.rearrange("p (h w) -> p h w", w=Wp)[:, :, 0:W]
                    eng = nc.sync if bi % 2 == 0 else nc.gpsimd
                    eng.dma_start(dst, x_ap[img, :, rb:rb + RB, :])
                    # duplicate this block one padded row up into the bottom
                    # half: bottom[j] = top[j + Wp] (contiguous, fast)
                    nc.scalar.dma_start(
                        x2[64:128, rb * Wp:(rb + RB) * Wp],
                        top[:, (rb + 1) * Wp:(rb + 1 + RB) * Wp])
                # tail of the bottom half (from top's zero tail / last pad row)
                nc.scalar.dma_start(x2[64:128, H * Wp:LB - Wp],
                                    top[:, (H + 1) * Wp:LB])

                qo = qp.tile([128, NCHQ * CH], f32)
                oimg = qo[0:64, 0:H * W]          # final-HBM-layout staging
                qbuf = qo[64:128, :]              # Q ring, padded b-space
                next_rb = 0
                psum_tiles = {}
                for c in range(NCHQ):
                    s = c * CH
                    ps = pp.tile([128, CH], f32)
                    for a in range(3):
                        nc.tensor.matmul(ps[:], w_sb[:, a * 128:(a + 1) * 128],
                                         x2[:, s + a:s + a + CH],
                                         start=(a == 0), stop=(a == 2))
                    # stage Q (psum bottom half) into the SBUF ring
                    nc.scalar.copy(qbuf[:, s:s + CH], ps[64:128, :])
                    if c >= 1:
                        cc = c - 1
                        sc = cc * CH
                        pprev = psum_tiles[cc % 6]
                        pv = pprev[0:64, :].rearrange("p (h w) -> p h w", w=Wp)[:, :, 0:W]
                        qv = qbuf[:, sc + 2 * Wp:sc + 2 * Wp + CH]
                        qv = qv.rearrange("p (h w) -> p h w", w=Wp)[:, :, 0:W]
                        ov = oimg[:, cc * RPC * W:(cc + 1) * RPC * W]
                        nc.vector.scalar_tensor_tensor(
                            ov.rearrange("p (h w) -> p h w", w=W),
                            pv, bias_sb[64:128, 0:1], qv, ALU.add, ALU.add)
                        done_rows = (cc + 1) * RPC
                        while next_rb < H and next_rb + RB <= done_rows:
                            src = oimg[:, W * next_rb:W * (next_rb + RB)]
                            nc.scalar.dma_start(
                                y_ap[img, :, next_rb:next_rb + RB, :],
                                src.rearrange("p (h w) -> p h w", w=W))
                            next_rb += RB
                    psum_tiles[c % 6] = ps

    nc.compile()
    nc.m = get_hw_module(nc.m)
    return nc


def _get_module():
    if "nc" not in _CACHE:
        _CACHE["nc"] = _build_module()
    return _CACHE["nc"]


def _make_in_maps(x, weight, bias):
    weight = np.asarray(weight, np.float32)
    wt = np.zeros((3, 128, 128), np.float32)
    for a in range(3):
        wt[a, 0:64, 0:64] = weight[:, :, 0, a].T    # P <- row-0 tap
        wt[a, 64:128, 0:64] = weight[:, :, 1, a].T  # P <- row-1 tap (shifted half)
        wt[a, 0:64, 64:128] = weight[:, :, 2, a].T  # Q <- row-2 tap
    bias2 = np.tile(np.asarray(bias, np.float32).reshape(COUT, 1), (2, 1))
    x = np.asarray(x, np.float32)
    return [{"x": np.ascontiguousarray(x[c * IPC:(c + 1) * IPC]),
             "wt": wt, "bias2": bias2} for c in range(N_CORES)]


def _run(in_maps, trace=False):
    from concourse import bass_utils
    nc = _get_module()
    return bass_utils.run_bass_kernel_spmd(
        nc, in_maps, core_ids=list(range(N_CORES)), trace=trace)


def kernel(x, weight, bias):
    res = _run(_make_in_maps(x, weight, bias), trace=False)
    return np.concatenate([res.results[c]["y"] for c in range(N_CORES)], axis=0)


# revision 14
# speedup vs baseline: 1.0790x; 1.0790x over previous
"""Trainium2 Bass kernel for nn_Conv2d (B=32, Cin=Cout=64, H=W=112, 3x3, pad 1).

Strategy (tap-packed, full 128x128 PE utilization):
- Data-parallel: 32 images / 8 cores = 4 images per core; weights/bias replicated.
- Per image: SBUF partitions 0-63 hold the zero-padded (114x114, flattened) image;
  partitions 64-127 hold the SAME image shifted down one padded row (built by
  contiguous SBUF->SBUF DMAs). A matmul with K=128 then sees two vertically
  adjacent padded rows of the image at once.
- Per 456-element chunk (4 padded rows) only THREE matmuls (horizontal taps
  a=0,1,2), each with stationary lhsT [128,128]:
      [[w(0,a)^T, w(2,a)^T], [w(1,a)^T, 0]]
  PSUM partitions 0-63 accumulate P = rows-0/1 taps; partitions 64-127
  accumulate Q = row-2 taps on the unshifted grid. Chunks are processed two at
  a time into a 2-bank PSUM tile so epilogue ops cover 912 elements each.
- Epilogue per double-chunk: ScalarE copies Q into a linear SBUF ring (Qbuf);
  VectorE computes out = (P + bias) + Q[b + 2*114] with one fused
  scalar_tensor_tensor, dropping pad columns so the staging buffer is in final
  HBM layout; outputs stream out as contiguous row-block DMAs.
- dtype float32r: fp32 storage, fast PE mode (1 cycle/row at N>=256),
  ~1.4e-4 max relative error end-to-end.
"""
import numpy as np

B, CIN, COUT, H, W = 32, 64, 64, 112, 112
N_CORES = 8
IPC = B // N_CORES          # images per core = 4
Wp = W + 2                  # padded width 114
Hp = H + 2                  # padded height 114
RPC = 4                     # rows per sub-chunk
CH = RPC * Wp               # sub-chunk size 456 (fits one PSUM bank)
NDC = 15                    # double-chunks; last one only stages Q (29 sub-chunks)
LB = 13440                  # padded-image buffer length (with tail margin)
RB = 28                     # rows per input/output DMA block

_CACHE = {}


def _build_module():
    import concourse.tile as tile
    from concourse import bacc, mybir
    from concourse.bass_interp import get_hw_module

    f32 = mybir.dt.float32
    f32r = mybir.dt.float32r
    ALU = mybir.AluOpType

    nc = bacc.Bacc("TRN2", target_bir_lowering=False, debug=False,
                   enable_asserts=False, num_devices=N_CORES)
    x_ap = nc.dram_tensor("x", [IPC, CIN, H, W], f32r, kind="ExternalInput").ap()
    wt_ap = nc.dram_tensor("wt", [3, 128, 128], f32r, kind="ExternalInput").ap()
    b_ap = nc.dram_tensor("bias2", [128, 1], f32, kind="ExternalInput").ap()
    y_ap = nc.dram_tensor("y", [IPC, COUT, H, W], f32, kind="ExternalOutput").ap()

    with tile.TileContext(nc) as tc:
        with (
            tc.tile_pool(name="const", bufs=1) as cp,
            tc.tile_pool(name="x2", bufs=2) as xp,
            tc.tile_pool(name="qo", bufs=1) as qp,
            tc.tile_pool(name="psum", bufs=4, space="PSUM") as pp,
        ):
            w_sb = cp.tile([128, 3 * 128], f32r)
            nc.sync.dma_start(w_sb[:].rearrange("k (t m) -> k t m", t=3),
                              wt_ap.rearrange("t k m -> k t m"))
            bias_sb = cp.tile([128, 1], f32)
            nc.sync.dma_start(bias_sb[:], b_ap[:])

            for img in range(IPC):
                x2 = xp.tile([128, LB], f32r)
                # zero pad borders on the top half (interior comes from DMA;
                # bottom half is duplicated from the top after)
                top = x2[0:64, :]
                nc.vector.memset(top[:, 0:Wp].bitcast(f32), 0.0)           # pad row 0
                nc.vector.memset(top[:, (Hp - 1) * Wp:LB].bitcast(f32), 0.0)  # row113+tail
                interior = top[:, Wp:Wp + H * Wp].rearrange("p (h w) -> p h w", w=Wp)
                nc.vector.memset(interior[:, :, 0:1].bitcast(f32), 0.0)    # left pad col
                nc.vector.memset(interior[:, :, Wp - 1:Wp].bitcast(f32), 0.0)  # right pad
                # interior row-blocks + matching one-row-up duplicates,
                # spread across the Sync and GpSimd DMA queues
                for bi, rb in enumerate(range(0, H, RB)):
                    lo = Wp * (1 + rb) + 1
                    dst = top[:, lo:lo + RB * Wp]
                    dst = dst.rearrange("p (h w) -> p h w", w=Wp)[:, :, 0:W]
                    eng, eng2 = (nc.sync, nc.gpsimd) if bi % 2 == 0 else (nc.gpsimd, nc.sync)
                    eng.dma_start(dst, x_ap[img, :, rb:rb + RB, :])
                    # bottom[j] = top[j + Wp] (contiguous, fast)
                    eng2.dma_start(x2[64:128, rb * Wp:(rb + RB) * Wp],
                                   top[:, (rb + 1) * Wp:(rb + 1 + RB) * Wp])
                # tail of the bottom half (from top's zero tail / last pad row)
                nc.sync.dma_start(x2[64:128, H * Wp:LB - Wp],
                                  top[:, (H + 1) * Wp:LB])

                qo = qp.tile([128, NDC * 2 * CH], f32)
                oimg = qo[0:64, 0:H * W]          # final-HBM-layout staging
                qbuf = qo[64:128, :]              # Q, linear padded b-space
                next_rb = 0
                psum_tiles = {}
                for dc in range(NDC):
                    ps = pp.tile([128, 1024], f32)
                    nhalf = 1 if dc == NDC - 1 else 2
                    for half in range(nhalf):
                        s = (2 * dc + half) * CH
                        reg = ps[:, half * 512:half * 512 + CH]
                        for a in range(3):
                            nc.tensor.matmul(reg, w_sb[:, a * 128:(a + 1) * 128],
                                             x2[:, s + a:s + a + CH],
                                             start=(a == 0), stop=(a == 2))
                    # stage Q (psum bottom half) into the linear SBUF buffer
                    qsrc = ps[64:128, 0:512 * nhalf]
                    qsrc = qsrc.rearrange("p (h w) -> p h w", w=512)[:, :, 0:CH]
                    qdst = qbuf[:, dc * 2 * CH:dc * 2 * CH + nhalf * CH]
                    nc.scalar.copy(qdst.rearrange("p (h w) -> p h w", w=CH), qsrc)
                    psum_tiles[dc] = ps
                    # combines for sub-chunks whose Q range is now staged:
                    # c = 2*dc-1 (prev tile bank 1) and c = 2*dc (this bank 0)
                    for c in (2 * dc - 1, 2 * dc):
                        if c < 0 or c > 27:
                            continue
                        cdc, bank = divmod(c, 2)
                        pt = psum_tiles[cdc]
                        pv = pt[0:64, bank * 512:bank * 512 + CH]
                        pv = pv.rearrange("p (h w) -> p h w", w=Wp)[:, :, 0:W]
                        sc = c * CH
                        qv = qbuf[:, sc + 2 * Wp:sc + 2 * Wp + CH]
                        qv = qv.rearrange("p (h w) -> p h w", w=Wp)[:, :, 0:W]
                        ov = oimg[:, c * RPC * W:(c + 1) * RPC * W]
                        nc.vector.scalar_tensor_tensor(
                            ov.rearrange("p (h w) -> p h w", w=W),
                            pv, bias_sb[64:128, 0:1], qv, ALU.add, ALU.add)
                        if bank == 1:
                            psum_tiles.pop(cdc)
                        done_rows = (c + 1) * RPC
                        while next_rb < H and next_rb + RB <= done_rows:
                            src = oimg[:, W * next_rb:W * (next_rb + RB)]
                            nc.scalar.dma_start(
                                y_ap[img, :, next_rb:next_rb + RB, :],
                                src.rearrange("p (h w) -> p h w", w=W))
                            next_rb += RB

    nc.compile()
    nc.m = get_hw_module(nc.m)
    return nc


def _get_module():
    if "nc" not in _CACHE:
        _CACHE["nc"] = _build_module()
    return _CACHE["nc"]


def _make_in_maps(x, weight, bias):
    weight = np.asarray(weight, np.float32)
    wt = np.zeros((3, 128, 128), np.float32)
    for a in range(3):
        wt[a, 0:64, 0:64] = weight[:, :, 0, a].T    # P <- row-0 tap
        wt[a, 64:128, 0:64] = weight[:, :, 1, a].T  # P <- row-1 tap (shifted half)
        wt[a, 0:64, 64:128] = weight[:, :, 2, a].T  # Q <- row-2 tap
    bias2 = np.tile(np.asarray(bias, np.float32).reshape(COUT, 1), (2, 1))
    x = np.asarray(x, np.float32)
    return [{"x": np.ascontiguousarray(x[c * IPC:(c + 1) * IPC]),
             "wt": wt, "bias2": bias2} for c in range(N_CORES)]


def _run(in_maps, trace=False):
    from concourse import bass_utils
    nc = _get_module()
    return bass_utils.run_bass_kernel_spmd(
        nc, in_maps, core_ids=list(range(N_CORES)), trace=trace)


def kernel(x, weight, bias):
    res = _run(_make_in_maps(x, weight, bias), trace=False)
    return np.concatenate([res.results[c]["y"] for c in range(N_CORES)], axis=0)


# revision 16
# speedup vs baseline: 1.1420x; 1.0585x over previous
"""Trainium2 Bass kernel for nn_Conv2d (B=32, Cin=Cout=64, H=W=112, 3x3, pad 1).

Strategy (tap-packed, full 128x128 PE utilization):
- Data-parallel: 32 images / 8 cores = 4 images per core; weights/bias replicated.
- Per image: SBUF partitions 0-63 hold the zero-padded (114x114, flattened) image;
  partitions 64-127 hold the SAME image shifted down one padded row (built by
  contiguous SBUF->SBUF DMAs). A matmul with K=128 then sees two vertically
  adjacent padded rows of the image at once.
- Per 456-element chunk (4 padded rows) only THREE matmuls (horizontal taps
  a=0,1,2), each with stationary lhsT [128,128]:
      [[w(0,a)^T, w(2,a)^T], [w(1,a)^T, 0]]
  PSUM partitions 0-63 accumulate P = rows-0/1 taps; partitions 64-127
  accumulate Q = row-2 taps on the unshifted grid. Chunks are processed two at
  a time into a 2-bank PSUM tile so epilogue ops cover 912 elements each.
- Epilogue per double-chunk: ScalarE copies Q into a linear SBUF ring (Qbuf);
  VectorE computes out = (P + bias) + Q[b + 2*114] with one fused
  scalar_tensor_tensor, dropping pad columns so the staging buffer is in final
  HBM layout; outputs stream out as contiguous row-block DMAs.
- dtype float32r: fp32 storage, fast PE mode (1 cycle/row at N>=256),
  ~1.4e-4 max relative error end-to-end.
"""
import numpy as np

B, CIN, COUT, H, W = 32, 64, 64, 112, 112
N_CORES = 8
IPC = B // N_CORES          # images per core = 4
Wp = W + 2                  # padded width 114
Hp = H + 2                  # padded height 114
RPC = 4                     # rows per sub-chunk
CH = RPC * Wp               # sub-chunk size 456 (fits one PSUM bank)
NDC = 15                    # double-chunks; last one only stages Q (29 sub-chunks)
LB = 13440                  # padded-image buffer length (with tail margin)
RB = 28                     # rows per input/output DMA block

_CACHE = {}


def _build_module():
    import concourse.tile as tile
    from concourse import bacc, mybir
    from concourse.bass_interp import get_hw_module

    f32 = mybir.dt.float32
    f32r = mybir.dt.float32r
    ALU = mybir.AluOpType

    nc = bacc.Bacc("TRN2", target_bir_lowering=False, debug=False,
                   enable_asserts=False, num_devices=N_CORES)
    x_ap = nc.dram_tensor("x", [IPC, CIN, H, W], f32r, kind="ExternalInput").ap()
    wt_ap = nc.dram_tensor("wt", [3, 128, 128], f32r, kind="ExternalInput").ap()
    b_ap = nc.dram_tensor("bias2", [128, 1], f32, kind="ExternalInput").ap()
    y_ap = nc.dram_tensor("y", [IPC, COUT, H, W], f32, kind="ExternalOutput").ap()

    with tile.TileContext(nc) as tc:
        with (
            tc.tile_pool(name="const", bufs=1) as cp,
            tc.tile_pool(name="psum", bufs=4, space="PSUM") as pp,
        ):
            w_sb = cp.tile([128, 3 * 128], f32r)
            nc.sync.dma_start(w_sb[:].rearrange("k (t m) -> k t m", t=3),
                              wt_ap.rearrange("t k m -> k t m"))
            bias_sb = cp.tile([128, 1], f32)
            nc.sync.dma_start(bias_sb[:], b_ap[:])

            # persistent buffers, reused across images with manual parity:
            # range-granular dependency tracking then lets image i+2's loads
            # start as soon as image i's early chunks are consumed (a pooled
            # slot would serialize on the image's LAST access instead)
            x2s = [cp.tile([128, LB], f32r, name=f"x2_{k}", tag=f"x2_{k}")
                   for k in range(2)]
            qo = cp.tile([128, NDC * 2 * CH], f32)
            oimg = qo[0:64, 0:H * W]              # final-HBM-layout staging
            qbuf = qo[64:128, :]                  # Q, linear padded b-space

            for x2 in x2s:
                # pad borders are zero for every image: write them ONCE
                top = x2[0:64, :]
                nc.vector.memset(top[:, 0:Wp].bitcast(f32), 0.0)           # pad row 0
                nc.vector.memset(top[:, (Hp - 1) * Wp:LB].bitcast(f32), 0.0)  # row113+tail
                interior = top[:, Wp:Wp + H * Wp].rearrange("p (h w) -> p h w", w=Wp)
                nc.vector.memset(interior[:, :, 0:1].bitcast(f32), 0.0)    # left pad col
                nc.vector.memset(interior[:, :, Wp - 1:Wp].bitcast(f32), 0.0)  # right pad
                # bottom-half tail = top's zero tail: also constant, copy once
                nc.sync.dma_start(x2[64:128, H * Wp:LB - Wp],
                                  top[:, (H + 1) * Wp:LB])

            for img in range(IPC):
                x2 = x2s[img % 2]
                top = x2[0:64, :]
                # interior row-blocks + matching one-row-up duplicates,
                # spread across the Sync and GpSimd DMA queues
                for bi, rb in enumerate(range(0, H, RB)):
                    lo = Wp * (1 + rb) + 1
                    dst = top[:, lo:lo + RB * Wp]
                    dst = dst.rearrange("p (h w) -> p h w", w=Wp)[:, :, 0:W]
                    eng, eng2 = (nc.sync, nc.gpsimd) if bi % 2 == 0 else (nc.gpsimd, nc.sync)
                    eng.dma_start(dst, x_ap[img, :, rb:rb + RB, :])
                    # bottom[j] = top[j + Wp] (contiguous, fast)
                    eng2.dma_start(x2[64:128, rb * Wp:(rb + RB) * Wp],
                                   top[:, (rb + 1) * Wp:(rb + 1 + RB) * Wp])

                next_rb = 0
                psum_tiles = {}
                for dc in range(NDC):
                    ps = pp.tile([128, 1024], f32)
                    nhalf = 1 if dc == NDC - 1 else 2
                    for half in range(nhalf):
                        s = (2 * dc + half) * CH
                        reg = ps[:, half * 512:half * 512 + CH]
                        for a in range(3):
                            nc.tensor.matmul(reg, w_sb[:, a * 128:(a + 1) * 128],
                                             x2[:, s + a:s + a + CH],
                                             start=(a == 0), stop=(a == 2))
                    # stage Q (psum bottom half) into the linear SBUF buffer
                    qsrc = ps[64:128, 0:512 * nhalf]
                    qsrc = qsrc.rearrange("p (h w) -> p h w", w=512)[:, :, 0:CH]
                    qdst = qbuf[:, dc * 2 * CH:dc * 2 * CH + nhalf * CH]
                    nc.scalar.copy(qdst.rearrange("p (h w) -> p h w", w=CH), qsrc)
                    psum_tiles[dc] = ps
                    # combines for sub-chunks whose Q range is now staged:
                    # c = 2*dc-1 (prev tile bank 1) and c = 2*dc (this bank 0)
                    for c in (2 * dc - 1, 2 * dc):
                        if c < 0 or c > 27:
                            continue
                        cdc, bank = divmod(c, 2)
                        pt = psum_tiles[cdc]
                        pv = pt[0:64, bank * 512:bank * 512 + CH]
                        pv = pv.rearrange("p (h w) -> p h w", w=Wp)[:, :, 0:W]
                        sc = c * CH
                        qv = qbuf[:, sc + 2 * Wp:sc + 2 * Wp + CH]
                        qv = qv.rearrange("p (h w) -> p h w", w=Wp)[:, :, 0:W]
                        ov = oimg[:, c * RPC * W:(c + 1) * RPC * W]
                        nc.vector.scalar_tensor_tensor(
                            ov.rearrange("p (h w) -> p h w", w=W),
                            pv, bias_sb[64:128, 0:1], qv, ALU.add, ALU.add)
                        if bank == 1:
                            psum_tiles.pop(cdc)
                        done_rows = (c + 1) * RPC
                        while next_rb < H and next_rb + RB <= done_rows:
                            src = oimg[:, W * next_rb:W * (next_rb + RB)]
                            nc.scalar.dma_start(
                                y_ap[img, :, next_rb:next_rb + RB, :],
                                src.rearrange("p (h w) -> p h w", w=W))
                            next_rb += RB

    nc.compile()
    nc.m = get_hw_module(nc.m)
    return nc


def _get_module():
    if "nc" not in _CACHE:
        _CACHE["nc"] = _build_module()
    return _CACHE["nc"]


def _make_in_maps(x, weight, bias):
    weight = np.asarray(weight, np.float32)
    wt = np.zeros((3, 128, 128), np.float32)
    for a in range(3):
        wt[a, 0:64, 0:64] = weight[:, :, 0, a].T    # P <- row-0 tap
        wt[a, 64:128, 0:64] = weight[:, :, 1, a].T  # P <- row-1 tap (shifted half)
        wt[a, 0:64, 64:128] = weight[:, :, 2, a].T  # Q <- row-2 tap
    bias2 = np.tile(np.asarray(bias, np.float32).reshape(COUT, 1), (2, 1))
    x = np.asarray(x, np.float32)
    return [{"x": np.ascontiguousarray(x[c * IPC:(c + 1) * IPC]),
             "wt": wt, "bias2": bias2} for c in range(N_CORES)]


def _run(in_maps, trace=False):
    from concourse import bass_utils
    nc = _get_module()
    return bass_utils.run_bass_kernel_spmd(
        nc, in_maps, core_ids=list(range(N_CORES)), trace=trace)


def kernel(x, weight, bias):
    res = _run(_make_in_maps(x, weight, bias), trace=False)
    return np.concatenate([res.results[c]["y"] for c in range(N_CORES)], axis=0)


# revision 18
# speedup vs baseline: 1.6086x; 1.4085x over previous
"""Trainium2 Bass kernel for nn_Conv2d (B=32, Cin=Cout=64, H=W=112, 3x3, pad 1).

Strategy:
- Data-parallel: 32 images / 8 cores = 4 images per core; weights/bias replicated.
- Per core: process 2 image-PAIRS. Image A lives on SBUF partitions 0-63,
  image B on partitions 64-127, each zero-padded to 114x114 and flattened.
- Conv = 9 accumulating PE matmuls per 512-pixel chunk: for tap (r,c) the
  stationary lhsT is a 128x128 block-diagonal tile diag(w_rc^T, w_rc^T) so the
  two images convolve independently; the moving rhs is the padded image buffer
  at free-dim offset r*114+c. PSUM (fp32) accumulates all 9 taps.
- Epilogue: VectorE tensor_scalar_add(psum + per-partition bias) -> SBUF staging,
  then one strided DMA per image drops the pad columns.
- dtype float32r: fp32 storage, fast PE mode (~1 cycle/row at N=512),
  measured ~1.5e-4 max relative error end-to-end.
"""
import numpy as np

B, CIN, COUT, H, W = 32, 64, 64, 112, 112
N_CORES = 8
IPC = B // N_CORES          # images per core = 4
NPAIR = IPC // 2            # image pairs per core = 2
Wp = W + 2                  # padded width 114
Hp = H + 2                  # padded height 114
ROWS_PER_CHUNK = 4
CH = ROWS_PER_CHUNK * Wp    # chunk size 456 (row-aligned; fits one PSUM bank)
NCHUNK = H // ROWS_PER_CHUNK     # 28 chunks cover all 112 output rows exactly
LB = Hp * Wp + 256          # padded-image buffer length (+tail margin)

_CACHE = {}


def _build_module():
    import concourse.tile as tile
    from concourse import bacc, mybir
    from concourse.bass_interp import get_hw_module

    f32 = mybir.dt.float32
    f32r = mybir.dt.float32r

    nc = bacc.Bacc("TRN2", target_bir_lowering=False, debug=False,
                   enable_asserts=False, num_devices=N_CORES)
    x_ap = nc.dram_tensor("x", [IPC, CIN, H, W], f32r, kind="ExternalInput").ap()
    wt_ap = nc.dram_tensor("wt", [9, 128, 128], f32r, kind="ExternalInput").ap()
    b_ap = nc.dram_tensor("bias2", [128, 1], f32, kind="ExternalInput").ap()
    y_ap = nc.dram_tensor("y", [IPC, COUT, H, W], f32, kind="ExternalOutput").ap()

    with tile.TileContext(nc) as tc:
        with (
            tc.tile_pool(name="const", bufs=1) as cp,
            tc.tile_pool(name="psum", bufs=8, space="PSUM") as pp,
        ):
            # weights on the (initially idle) Scalar queue so the Sync/GpSimd
            # queues start streaming image data immediately
            w_sb = cp.tile([128, 9 * 128], f32r)
            nc.scalar.dma_start(w_sb[:].rearrange("k (t m) -> k t m", t=9),
                                wt_ap.rearrange("t k m -> k t m"))
            bias_sb = cp.tile([128, 1], f32)
            nc.scalar.dma_start(bias_sb[:], b_ap[:])

            # persistent buffers (one x2 per pair; one shared output staging):
            # range-granular deps let every load/evac start as early as its
            # data allows, with no pool-slot whole-tile serialization
            x2s = [cp.tile([128, LB], f32r, name=f"x2_{k}", tag=f"x2_{k}")
                   for k in range(NPAIR)]
            oimg = cp.tile([128, H * W], f32)

            # input row-blocks: small slivers first so chunk-0 matmuls start
            # almost immediately; alternate Sync/GpSimd DMA queues (one
            # queue's 448B-row rate is only ~180 GB/s; two together saturate
            # HBM)
            IN_BLOCKS = [(0, 7), (7, 7), (14, 14), (28, 28), (56, 28), (84, 28)]
            for p in range(NPAIR):
                x2 = x2s[p]
                # zero the pad borders (rest is overwritten by the image DMA);
                # memset doesn't accept f32r so bitcast the views to f32
                nc.vector.memset(x2[:, 0:Wp].bitcast(f32), 0.0)          # top pad row
                nc.vector.memset(x2[:, (Hp - 1) * Wp:LB].bitcast(f32), 0.0)  # bottom+tail
                interior = x2[:, Wp:Wp + H * Wp].rearrange("p (h w) -> p h w", w=Wp)
                nc.vector.memset(interior[:, :, 0:1].bitcast(f32), 0.0)  # left pad col
                nc.vector.memset(interior[:, :, Wp - 1:Wp].bitcast(f32), 0.0)  # right pad
                for rb, nr in IN_BLOCKS:
                    for h in range(2):
                        img = 2 * p + h
                        lo = Wp * (1 + rb) + 1
                        dst = x2[64 * h:64 * (h + 1), lo:lo + nr * Wp]
                        dst = dst.rearrange("p (h w) -> p h w", w=Wp)[:, :, 0:W]
                        eng = nc.sync if h == 0 else nc.gpsimd
                        eng.dma_start(dst, x_ap[img, :, rb:rb + nr, :])

            RB = 14  # rows per output DMA block
            for p in range(NPAIR):
                x2 = x2s[p]
                next_rb = 0
                for c in range(NCHUNK):
                    s = c * CH
                    ps = pp.tile([128, CH], f32)
                    for t in range(9):
                        r, cc = divmod(t, 3)
                        off = r * Wp + cc
                        nc.tensor.matmul(ps[:], w_sb[:, t * 128:(t + 1) * 128],
                                         x2[:, s + off:s + off + CH],
                                         start=(t == 0), stop=(t == 8))
                    # psum holds ROWS_PER_CHUNK padded rows; keep the 112 valid
                    # columns of each, add bias, write contiguous HBM layout
                    pv = ps[:].rearrange("p (h w) -> p h w", w=Wp)[:, :, 0:W]
                    ov = oimg[:, c * ROWS_PER_CHUNK * W:(c + 1) * ROWS_PER_CHUNK * W]
                    nc.vector.tensor_scalar_add(
                        ov.rearrange("p (h w) -> p h w", w=W), pv, bias_sb[:])
                    # emit output row-blocks as soon as their rows are evacuated
                    # (ScalarE's DMA queue, so they never block input loads)
                    while next_rb < H and (next_rb + RB) <= (c + 1) * ROWS_PER_CHUNK:
                        for h in range(2):
                            img = 2 * p + h
                            src = oimg[64 * h:64 * (h + 1),
                                       W * next_rb:W * (next_rb + RB)]
                            nc.scalar.dma_start(
                                y_ap[img, :, next_rb:next_rb + RB, :],
                                src.rearrange("p (h w) -> p h w", w=W))
                        next_rb += RB

    nc.compile()
    nc.m = get_hw_module(nc.m)
    return nc


def _get_module():
    if "nc" not in _CACHE:
        _CACHE["nc"] = _build_module()
    return _CACHE["nc"]


def _make_in_maps(x, weight, bias):
    wt = np.zeros((9, 128, 128), np.float32)
    for t in range(9):
        r, cc = divmod(t, 3)
        wT = np.ascontiguousarray(weight[:, :, r, cc].T)  # [cin, cout]
        wt[t, :64, :64] = wT
        wt[t, 64:, 64:] = wT
    bias2 = np.tile(np.asarray(bias, np.float32).reshape(COUT, 1), (2, 1))
    x = np.asarray(x, np.float32)
    return [{"x": np.ascontiguousarray(x[c * IPC:(c + 1) * IPC]),
             "wt": wt, "bias2": bias2} for c in range(N_CORES)]


def _run(in_maps, trace=False):
    from concourse import bass_utils
    nc = _get_module()
    return bass_utils.run_bass_kernel_spmd(
        nc, in_maps, core_ids=list(range(N_CORES)), trace=trace)


def kernel(x, weight, bias):
    res = _run(_make_in_maps(x, weight, bias), trace=False)
    return np.concatenate([res.results[c]["y"] for c in range(N_CORES)], axis=0)
